# revision 12
# baseline (speedup 1.0000x reference)
"""GQA attention kernel for Trainium2, 8-core SPMD.

Sharding: core = b*4 + kv  (B=2 batches x HKV=4 kv-heads = 8 cores).
Each core computes its batch's 4 query heads (one GQA group) end to end:
q/k/v projections, softmax(QK^T)V, and the row-parallel slice of o_proj.
Host sums the 4 partial o_proj outputs per batch (the "all-reduce").
"""
import sys

sys.path.insert(0, "/opt/trn_rl_repo")
from contextlib import ExitStack

import numpy as np
import concourse.bass as bass
import concourse.tile as tile
from concourse import bacc, mybir
from concourse import bass_utils
from concourse.masks import make_identity

F32 = mybir.dt.float32
F32R = mybir.dt.float32r
EXP = mybir.ActivationFunctionType.Exp

B, S, D = 2, 2048, 1024
HKV, R, HD = 4, 4, 64          # kv heads, q-heads per kv head, head dim
GQ = R * HD                    # 256 q-proj cols per core
SCALE = HD ** -0.5
NCORES = 8

_CACHE = {}


def _r(ap):
    return ap.bitcast(F32R)


def _build():
    nc = bacc.Bacc("TRN2", target_bir_lowering=False, debug=False,
                   enable_asserts=False, num_devices=1)
    x_d = nc.dram_tensor("x", (S, D), F32, kind="ExternalInput").ap()
    wq_d = nc.dram_tensor("wq", (D, GQ), F32, kind="ExternalInput").ap()
    wkv_d = nc.dram_tensor("wkv", (D, 2 * HD), F32, kind="ExternalInput").ap()
    wo_d = nc.dram_tensor("wo", (GQ, D), F32, kind="ExternalInput").ap()
    po_d = nc.dram_tensor("po", (S, D), F32, kind="ExternalOutput").ap()

    with tile.TileContext(nc) as tc, ExitStack() as ctx:
        P = ctx.enter_context(tc.tile_pool(name="persist", bufs=1))
        xload = ctx.enter_context(tc.tile_pool(name="xload", bufs=4))
        psA = ctx.enter_context(tc.tile_pool(name="psA", bufs=2, space="PSUM"))
        psU = ctx.enter_context(tc.tile_pool(name="psU", bufs=2, space="PSUM"))
        work = ctx.enter_context(tc.tile_pool(name="work", bufs=3))
        nrm = ctx.enter_context(tc.tile_pool(name="nrm", bufs=1))

        ident = P.tile([128, 128], F32, tag="ident", name="ident")
        make_identity(nc, ident[:])
        ones = P.tile([1, 64], F32R, tag="ones", name="ones")
        nc.gpsimd.memset(ones[:].bitcast(F32), 1.0)

        # ---- load weights ----
        wq_sb = [P.tile([128, GQ], F32R, tag=f"wq{k}", name=f"wq{k}") for k in range(8)]
        wkv_sb = [P.tile([128, 2 * HD], F32R, tag=f"wkv{k}", name=f"wkv{k}")
                  for k in range(8)]
        wo_sb = [P.tile([64, D], F32R, tag=f"wo{h}", name=f"wo{h}") for h in range(4)]
        for k in range(8):
            nc.sync.dma_start(wq_sb[k][:], wq_d[k * 128:(k + 1) * 128, :].bitcast(F32R))
            nc.sync.dma_start(wkv_sb[k][:], wkv_d[k * 128:(k + 1) * 128, :].bitcast(F32R))
        for h in range(4):
            nc.sync.dma_start(wo_sb[h][:], wo_d[h * 64:(h + 1) * 64, :].bitcast(F32R))

        # ---- x^T via PE transposes: xt[k] = (128 d, 2048 s) ----
        xt = [P.tile([128, S], F32R, tag=f"xt{k}", name=f"xt{k}") for k in range(8)]
        for sg in range(4):                       # groups of 4 s-tiles
            xl = []
            for j in range(4):
                t = xload.tile([128, D], F32, tag="xl", name="xl")
                st = sg * 4 + j
                nc.sync.dma_start(t[:], x_d[st * 128:(st + 1) * 128, :])
                xl.append(t)
            for k in range(8):
                ps = psA.tile([128, 1024], F32, tag="A", name="atile")
                for j in range(4):
                    nc.tensor.transpose(ps[:, j * 128:(j + 1) * 128],
                                        xl[j][:, k * 128:(k + 1) * 128],
                                        ident[:])
                nc.vector.tensor_copy(
                    xt[k][:, sg * 512:(sg + 1) * 512], ps[:, 0:512])

        # ---- projections (all outputs at base partition 0) ----
        # qth[h] = (64 q-dim, 2048 s);  kt = (64 k-dim, 2048 s)
        qth = [P.tile([64, S], F32R, tag=f"qth{h}", name=f"qth{h}") for h in range(4)]
        kt = P.tile([64, S], F32R, tag="kt", name="kt")
        for h in range(4):
            for half in range(2):
                ps = psU.tile([65, 1024], F32, tag="U", name="utile")
                for k in range(8):
                    for c in range(2):
                        off = half * 1024 + c * 512
                        nc.tensor.matmul(ps[0:64, c * 512:(c + 1) * 512],
                                         wq_sb[k][:, h * 64:(h + 1) * 64],
                                         xt[k][:, off:off + 512],
                                         start=(k == 0), stop=(k == 7))
                nc.vector.tensor_copy(qth[h][:, half * 1024:(half + 1) * 1024],
                                      ps[0:64, :])
        for half in range(2):
            ps = psU.tile([65, 1024], F32, tag="U", name="utile")
            for k in range(8):
                for c in range(2):
                    off = half * 1024 + c * 512
                    nc.tensor.matmul(ps[0:64, c * 512:(c + 1) * 512],
                                     wkv_sb[k][:, 0:64],
                                     xt[k][:, off:off + 512],
                                     start=(k == 0), stop=(k == 7))
            nc.vector.tensor_copy(kt[:, half * 1024:(half + 1) * 1024], ps[0:64, :])

        # ---- V' in natural layout: vp[st] = (128 keys, 65) with ones col ----
        vp = [P.tile([128, HD + 1], F32R, tag=f"vp{j}", name=f"vp{j}")
              for j in range(16)]
        for st in range(16):
            ps = psA.tile([128, 1024], F32, tag="A", name="atile")
            for k in range(8):
                nc.tensor.matmul(ps[:, 0:64],
                                 xt[k][:, st * 128:(st + 1) * 128],
                                 wkv_sb[k][:, 64:128],
                                 start=(k == 0), stop=(k == 7))
            nc.vector.tensor_copy(vp[st][:, 0:64], ps[:, 0:64])
            nc.gpsimd.memset(vp[st][:, 64:65].bitcast(F32), 1.0)

        # ---- attention + normalize: oth[h] = (64 d, 2048 s) ----
        oth = [P.tile([64, S], F32R, tag=f"oth{h}", name=f"oth{h}") for h in range(4)]
        for h in range(4):
            for ib in range(2):
                ut = psU.tile([65, 1024], F32, tag="U", name="utile")
                for jt in range(16):
                    at = psA.tile([128, 1024], F32, tag="A", name="atile")
                    for c in range(2):
                        off = ib * 1024 + c * 512
                        nc.tensor.matmul(at[:, c * 512:(c + 1) * 512],
                                         kt[:, jt * 128:(jt + 1) * 128],
                                         qth[h][:, off:off + 512],
                                         start=True, stop=True)
                    ea = work.tile([128, 1024], F32R, tag="ea", name="ea")
                    nc.scalar.activation(ea[:], at[:], EXP, scale=SCALE)
                    for c in range(2):
                        nc.tensor.matmul(ut[:, c * 512:(c + 1) * 512],
                                         vp[jt][:],
                                         ea[:, c * 512:(c + 1) * 512],
                                         start=(jt == 0), stop=(jt == 15),
                                         skip_group_check=True)
                # normalize: oth[h][:, ib*1024:+1024] = ut[0:64] / ut[64]
                rs = nrm.tile([1, 1024], F32R, tag="rs", name="rs")
                with nc.allow_low_precision(reason="f32r normalizer, 6e-5 rel"):
                    nc.vector.reciprocal(rs[:], ut[64:65, :])
                bc = psU.tile([65, 1024], F32, tag="U", name="utile")
                for c in range(2):
                    nc.tensor.matmul(bc[0:64, c * 512:(c + 1) * 512],
                                     ones[:], rs[:, c * 512:(c + 1) * 512],
                                     start=True, stop=True)
                bcs = nrm.tile([64, 1024], F32, tag="bc", name="bcs")
                nc.vector.tensor_copy(bcs[:], bc[0:64, :])
                nc.vector.tensor_mul(oth[h][:, ib * 1024:(ib + 1) * 1024],
                                     ut[0:64, :], bcs[:])

        # ---- o_proj: po[st] = sum_h oth[h][:, st].T @ wo[h] ----
        for st in range(16):
            ps = psA.tile([128, 1024], F32, tag="A", name="atile")
            for h in range(4):
                for c in range(2):
                    nc.tensor.matmul(ps[:, c * 512:(c + 1) * 512],
                                     oth[h][:, st * 128:(st + 1) * 128],
                                     wo_sb[h][:, c * 512:(c + 1) * 512],
                                     start=(h == 0), stop=(h == 3))
            ostage = work.tile([128, 1024], F32, tag="ea", name="ostage")
            nc.vector.tensor_copy(ostage[:], ps[:])
            nc.sync.dma_start(po_d[st * 128:(st + 1) * 128, :], ostage[:])

    nc.compile()
    return nc


def kernel(x, Wq, Wk, Wv, Wo):
    x = np.ascontiguousarray(x, dtype=np.float32)
    in_maps = []
    for core in range(NCORES):
        b, kv = core // HKV, core % HKV
        in_maps.append({
            "x": np.ascontiguousarray(x[b]),
            "wq": np.ascontiguousarray(Wq[:, kv * GQ:(kv + 1) * GQ], dtype=np.float32),
            "wkv": np.ascontiguousarray(
                np.concatenate([Wk[:, kv * HD:(kv + 1) * HD],
                                Wv[:, kv * HD:(kv + 1) * HD]], axis=1),
                dtype=np.float32),
            "wo": np.ascontiguousarray(Wo[kv * GQ:(kv + 1) * GQ, :], dtype=np.float32),
        })
    if "nc" not in _CACHE:
        _CACHE["nc"] = _build()
    res = bass_utils.run_bass_kernel_spmd(
        _CACHE["nc"], in_maps, core_ids=list(range(NCORES)), trace=False)
    out = np.zeros((B, S, D), dtype=np.float32)
    for core in range(NCORES):
        out[core // HKV] += res.results[core]["po"]
    return out
